# revision 1
# baseline (speedup 1.0000x reference)
"""DeepFM backbone on 8 TRN2 NeuronCores.

Sharding: batch 16384 -> 2048 per core. Packed embedding table
(emb_table ++ fm1_table -> [R, 17]) replicated on every core; per-core
gather via gpsimd indirect DMA (128 rows / instruction). DNN runs
feature-major after PE transposes; BatchNorm batch statistics are
combined across cores with two small AllReduces.
"""
import sys
sys.path.insert(0, '/opt/trn_rl_repo')
import numpy as np

import concourse.bass as bass
import concourse.bacc as bacc
import concourse.tile as tile
import concourse.mybir as mybir
from concourse.bass_utils import run_bass_kernel_spmd

# ---- problem constants (hardcoded per contract) ----
SPARSE_DIMS = [1000000, 100000, 100000, 10000, 10000, 10000, 1000, 1000, 1000,
               1000, 100, 100, 100, 100, 50, 50, 20, 20, 10, 10]
NS = 20
ND = 13
FEAT = 16
H1, H2 = 256, 128
B = 16384
N_CORES = 8
BC = B // N_CORES            # 2048 samples per core
P = 128
NT = BC // P                 # 16 batch tiles per core
R_TOTAL = int(np.sum(SPARSE_DIMS))   # 1234560
OFFSETS = np.concatenate([[0], np.cumsum(SPARSE_DIMS)[:-1]]).astype(np.int32)
BN_EPS = 1e-5
F32 = mybir.dt.float32
I32 = mybir.dt.int32
AF = mybir.ActivationFunctionType
ALU = mybir.AluOpType

# feature chunking of the 528-dim DNN input:
# chunk 0: sparse fields 0-7   (128 features)
# chunk 1: sparse fields 8-15  (128 features)
# chunk 2: sparse fields 16-19 (64 features)
# chunk 3: dense fields 0-7    (128 features)
# chunk 4: dense fields 8-12   (80 features)
CHUNKS = [
    ("sp", 0, 8, 128),
    ("sp", 8, 16, 128),
    ("sp", 16, 20, 64),
    ("de", 0, 8, 128),
    ("de", 8, 13, 80),
]

_CACHE = {}


def _build(reps=1, skip_gather=False, skip_cc=False, all_indirect=False, tweak=True):
    nc = bacc.Bacc("TRN2", target_bir_lowering=False, debug=False,
                   num_devices=N_CORES)
    # ---- DRAM I/O ----
    xs = nc.dram_tensor("xs", [BC, NS + ND], F32, kind="ExternalInput")
    tbl = nc.dram_tensor("tbl", [R_TOTAL, FEAT + 1], F32, kind="ExternalInput")
    offs = nc.dram_tensor("offs", [P, NS], I32, kind="ExternalInput")
    w1 = nc.dram_tensor("w1", [5, P, H1], F32, kind="ExternalInput")   # padded chunks
    w2 = nc.dram_tensor("w2", [2, P, H2], F32, kind="ExternalInput")
    w3 = nc.dram_tensor("w3", [P, 1], F32, kind="ExternalInput")
    # per-feature vectors, feature-major [P, m]
    vec1 = nc.dram_tensor("vec1", [P, 2 * 4], F32, kind="ExternalInput")  # g1,be1,b1 x2 tiles + pad
    vec2 = nc.dram_tensor("vec2", [P, 4], F32, kind="ExternalInput")      # g2,be2,b2,pad
    dvec = nc.dram_tensor("dvec", [P, 2 * ND * FEAT + ND + 2], F32, kind="ExternalInput")
    iota_d = nc.dram_tensor("iota_d", [P, 1], F32, kind="ExternalInput")
    ones_d = nc.dram_tensor("ones_d", [1, P], F32, kind="ExternalInput")
    out = nc.dram_tensor("out", [P, NT], F32, kind="ExternalOutput")

    with tile.TileContext(nc) as tc:
        with tc.tile_pool(name="const", bufs=1) as cp, \
             tc.tile_pool(name="hsp", bufs=(6 if tweak else 4)) as hp, \
             tc.tile_pool(name="scratch", bufs=(6 if tweak else 4)) as sp, \
             tc.tile_pool(name="big", bufs=1) as bigp, \
             tc.tile_pool(name="ps_t", bufs=1, space="PSUM") as ps_t, \
             tc.tile_pool(name="ps_g", bufs=2, space="PSUM") as ps_g, \
             tc.tile_pool(name="ps_z", bufs=1, space="PSUM") as ps_z, \
             tc.tile_pool(name="ps_s", bufs=1, space="PSUM") as ps_s, \
             tc.tile_pool(name="dram", bufs=1, space="DRAM") as dp:

            # ---- load constants ----
            x_t = cp.tile([P, NT, NS + ND], F32)
            nc.sync.dma_start(out=x_t[:], in_=xs[:].rearrange("(t p) d -> p t d", p=P))
            off_t = cp.tile([P, NS], I32)
            nc.sync.dma_start(out=off_t[:], in_=offs[:])
            w1_t = cp.tile([P, 5, H1], F32)
            nc.sync.dma_start(out=w1_t[:], in_=w1[:].rearrange("k p n -> p k n"))
            w2_t = cp.tile([P, 2, H2], F32)
            nc.sync.dma_start(out=w2_t[:], in_=w2[:].rearrange("k p n -> p k n"))
            w3_t = cp.tile([P, 1], F32)
            nc.sync.dma_start(out=w3_t[:], in_=w3[:])
            vec1_t = cp.tile([P, 8], F32)
            nc.sync.dma_start(out=vec1_t[:], in_=vec1[:])
            vec2_t = cp.tile([P, 4], F32)
            nc.sync.dma_start(out=vec2_t[:], in_=vec2[:])
            dv_t = cp.tile([P, 2 * ND * FEAT + ND + 2], F32)
            nc.sync.dma_start(out=dv_t[:], in_=dvec[:])
            dw = dv_t[:, 0:ND * FEAT]                      # dense_w flat [1,208]
            db = dv_t[:, ND * FEAT:2 * ND * FEAT]          # dense_b flat
            dfw = dv_t[:, 2 * ND * FEAT:2 * ND * FEAT + ND]  # dense_fm_w [1,13]
            dfb_b3 = dv_t[:, 2 * ND * FEAT + ND:2 * ND * FEAT + ND + 1]  # sum(dense_fm_b)+b3
            identity = cp.tile([P, P], F32)
            from concourse.masks import make_identity
            make_identity(nc, identity[:])
            iota_t = cp.tile([P, 1], F32)
            nc.sync.dma_start(out=iota_t[:], in_=iota_d[:])
            ones_t = cp.tile([1, P], F32)
            nc.sync.dma_start(out=ones_t[:], in_=ones_d[:])
            SMALLF = list(range(10, 20))
            tblf = []
            for f in SMALLF:
                v = SPARSE_DIMS[f]
                tf = cp.tile([v, FEAT + 1], F32, name=f"tblf{f}")
                nc.sync.dma_start(
                    out=tf[:], in_=tbl[int(OFFSETS[f]):int(OFFSETS[f]) + v])
                tblf.append(tf)

            # ---- integer global row ids ----
            for _rep in range(reps):
                gid = cp.tile([P, NT, NS], I32)
                xi = cp.tile([P, NT, NS], I32)
                nc.vector.tensor_copy(xi[:], x_t[:, :, 0:NS])   # f32 -> i32 cast
                for t in range(NT):
                    nc.vector.tensor_tensor(
                        out=gid[:, t, :], in0=xi[:, t, :],
                        in1=off_t[:], op=ALU.add)

                # ---- feature-major activation buffers ----
                hT = [bigp.tile([P, BC], F32, name=f"hT{i}", tag=f"hT{i}") for i in range(5)]
                z1T = [bigp.tile([P, BC], F32, name=f"z1T{i}", tag=f"z1T{i}") for i in range(2)]
                z2T = bigp.tile([P, BC], F32)
                sq_scr = bigp.tile([P, BC], F32)    # scratch for square outputs
                fm_sb = bigp.tile([P, NT], F32)     # fm1+fm2 per tile
                out_sb = bigp.tile([P, NT], F32)
                stat1 = bigp.tile([P, 8], F32)      # z1 local sums: [m,{s,sq}] + pad
                stat2 = bigp.tile([P, 4], F32)

                # collective bounce buffers
                cc1_in = dp.tile([P, 4], F32)
                cc1_out = dp.tile([P, 4], F32)
                cc2_in = dp.tile([P, 2], F32)
                cc2_out = dp.tile([P, 2], F32)

                # ---- per-tile: gather + dense + FM + transposes ----
                for t in range(NT):
                    hsp = hp.tile([P, NS, FEAT + 1], F32, tag="hsp")
                    if skip_gather:
                        nc.vector.memset(hsp[:, 0:10, :], 0.01)
                    else:
                        for f in range(NS if all_indirect else 10):
                            nc.gpsimd.indirect_dma_start(
                                out=hsp[:, f, :], out_offset=None, in_=tbl[:],
                                in_offset=bass.IndirectOffsetOnAxis(
                                    ap=gid[:, t, f:f + 1], axis=0))
                    # small fields via one-hot matmul gather
                    for j, f in (() if all_indirect else list(enumerate(SMALLF))):
                        v = SPARSE_DIMS[f]
                        xiTp = ps_t.tile([1, P], F32, tag="xiTp", bufs=1)
                        xsrc = bass.AP(x_t.tensor, x_t[:, t, 10 + j:11 + j].offset,
                                       [x_t[:].ap[0], (1, 1)])
                        nc.tensor.transpose(out=xiTp[:], in_=xsrc,
                                            identity=identity[:])
                        xr = sp.tile([1, P], F32, tag="xr", bufs=4)
                        nc.vector.tensor_copy(out=xr[:], in_=xiTp[:])
                        idxb = ps_g.tile([P, P], F32, tag="idxb", bufs=2)
                        nc.tensor.matmul(out=idxb[:], lhsT=ones_t[:],
                                         rhs=xr[:], start=True, stop=True)
                        oh = sp.tile([P, P], F32, tag="oh")
                        nc.vector.tensor_tensor(
                            out=oh[:], in0=iota_t[:].to_broadcast([P, P]),
                            in1=idxb[:], op=ALU.is_equal)
                        ge = ps_g.tile([P, FEAT + 1], F32, tag="ge", bufs=2)
                        nc.tensor.matmul(out=ge[:], lhsT=oh[0:v, :],
                                         rhs=tblf[j][:], start=True, stop=True)
                        nc.scalar.activation(out=hsp[:, f, :], in_=ge[:], func=AF.Copy)

                    de = sp.tile([P, ND, FEAT], F32, tag="de")
                    # de = x_dense[:,:,None]*dense_w + dense_b
                    x3 = bass.AP(x_t.tensor, x_t[:, t, NS:NS + ND].offset,
                                 [x_t[:].ap[0], (1, ND), (0, FEAT)])
                    dw3 = bass.AP(dv_t.tensor, dw.offset,
                                  [dw.ap[0], (FEAT, ND), (1, FEAT)])
                    db3 = bass.AP(dv_t.tensor, db.offset,
                                  [db.ap[0], (FEAT, ND), (1, FEAT)])
                    nc.vector.tensor_tensor(out=de[:], in0=x3, in1=dw3, op=ALU.mult)
                    nc.vector.tensor_tensor(out=de[:], in0=de[:], in1=db3, op=ALU.add)

                    # compact emb features (drop fm1 col) for transposes/squares
                    hcomp = sp.tile([P, NS * FEAT], F32, tag="hcomp")
                    hcomp_v = bass.AP(hcomp.tensor, hcomp[:].offset,
                                      [hcomp[:].ap[0], (FEAT, NS), (1, FEAT)])
                    hsp_v = bass.AP(hsp.tensor, hsp[:].offset,
                                    [hsp[:].ap[0], (FEAT + 1, NS), (1, FEAT)])
                    nc.vector.tensor_copy(out=hcomp_v, in_=hsp_v)

                    # --- FM terms (batch-major) ---
                    s17 = sp.tile([P, FEAT + 1], F32, tag="s17")
                    # sum over sparse fields: view [P, 17, 20] (d outer stride 1, f inner stride 17)
                    hsp_dT = bass.AP(hsp.tensor, hsp[:].offset,
                                     [hsp[:].ap[0], (1, FEAT + 1), (FEAT + 1, NS)])
                    nc.vector.tensor_reduce(out=s17[:], in_=hsp_dT,
                                            axis=mybir.AxisListType.X, op=ALU.add)
                    s16 = sp.tile([P, FEAT], F32, tag="s16")
                    de_dT = bass.AP(de.tensor, de[:].offset,
                                    [de[:].ap[0], (1, FEAT), (FEAT, ND)])
                    nc.vector.tensor_reduce(out=s16[:], in_=de_dT,
                                            axis=mybir.AxisListType.X, op=ALU.add)
                    nc.vector.tensor_tensor(out=s16[:], in0=s16[:],
                                            in1=s17[:, 0:FEAT], op=ALU.add)
                    # sum of squares over all fields+features
                    sqs = sp.tile([P, 2], F32, tag="sqs")
                    hsp_sq = sp.tile([P, NS * FEAT], F32, tag="hsp_sq")
                    nc.scalar.activation(out=hsp_sq[:], in_=hcomp[:],
                                         func=AF.Square, accum_out=sqs[:, 0:1])
                    de_sq = sp.tile([P, ND, FEAT], F32, tag="de_sq")
                    nc.scalar.activation(out=de_sq[:], in_=de[:],
                                         func=AF.Square, accum_out=sqs[:, 1:2])
                    s2 = sp.tile([P, 1], F32, tag="s2")
                    s16sq = sp.tile([P, FEAT], F32, tag="s16sq")
                    nc.scalar.activation(out=s16sq[:], in_=s16[:],
                                         func=AF.Square, accum_out=s2[:])
                    # fm2 = 0.5*(s2 - sqs0 - sqs1)
                    fmv = sp.tile([P, 1], F32, tag="fmv")
                    nc.vector.tensor_tensor(out=fmv[:], in0=sqs[:, 0:1],
                                            in1=sqs[:, 1:2], op=ALU.add)
                    nc.vector.tensor_tensor(out=fmv[:], in0=s2[:], in1=fmv[:],
                                            op=ALU.subtract)
                    # dense fm1: sum(x_de * dfw) ; plus consts sum(dense_fm_b)+b3
                    dfm = sp.tile([P, ND], F32, tag="dfm")
                    nc.vector.tensor_tensor(
                        out=dfm[:], in0=x_t[:, t, NS:NS + ND],
                        in1=dfw, op=ALU.mult)
                    dfm1 = sp.tile([P, 1], F32, tag="dfm1")
                    nc.vector.tensor_reduce(out=dfm1[:], in_=dfm[:],
                                            axis=mybir.AxisListType.X, op=ALU.add)
                    # fm_sb[:, t] = 0.5*fmv + s17[:,16] + dfm1 + (sum dfb + b3)
                    nc.vector.tensor_scalar(out=fmv[:], in0=fmv[:], scalar1=0.5,
                                            scalar2=None, op0=ALU.mult)
                    nc.vector.tensor_tensor(out=fmv[:], in0=fmv[:],
                                            in1=s17[:, FEAT:FEAT + 1], op=ALU.add)
                    nc.vector.tensor_tensor(out=fmv[:], in0=fmv[:], in1=dfm1[:],
                                            op=ALU.add)
                    nc.vector.tensor_tensor(
                        out=fm_sb[:, t:t + 1], in0=fmv[:],
                        in1=dfb_b3, op=ALU.add)

                    # --- transposes into feature-major hT ---
                    for k, (kind, f0, f1, nf) in enumerate(CHUNKS):
                        pt = ps_t.tile([P, P], F32, tag="pt")
                        if kind == "sp":
                            tsrc = bass.AP(hcomp.tensor, hcomp[:, f0 * FEAT:f1 * FEAT].offset,
                                           [hcomp[:].ap[0], (1, nf)])
                        else:
                            tsrc = bass.AP(de.tensor, de[:, f0:f1, :].offset,
                                           [de[:].ap[0], (1, nf)])
                        nc.tensor.transpose(out=pt[0:nf, 0:P], in_=tsrc,
                                            identity=identity[:])
                        if tweak:
                            nc.scalar.activation(
                                out=hT[k][0:nf, t * P:(t + 1) * P],
                                in_=pt[0:nf, 0:P], func=AF.Copy)
                        else:
                            nc.vector.tensor_copy(
                                out=hT[k][0:nf, t * P:(t + 1) * P],
                                in_=pt[0:nf, 0:P])

                # ---- layer 1: z1^T = W1^T @ h^T, feature-major ----
                NGR = 4
                GW = BC // NGR   # 512
                for m in range(2):
                    for g in range(NGR):
                        pz = ps_z.tile([P, GW], F32, tag="pz")
                        for k in range(5):
                            nf = CHUNKS[k][3]
                            nc.tensor.matmul(
                                out=pz[:], lhsT=w1_t[0:nf, k, m * P:(m + 1) * P],
                                rhs=hT[k][0:nf, g * GW:(g + 1) * GW],
                                start=(k == 0), stop=(k == 4))
                        nc.vector.tensor_copy(out=z1T[m][:, g * GW:(g + 1) * GW], in_=pz[:])
                    # local stats
                    nc.vector.tensor_reduce(out=stat1[:, 4 * m:4 * m + 1],
                                            in_=z1T[m][:], axis=mybir.AxisListType.X,
                                            op=ALU.add)
                    nc.scalar.activation(out=sq_scr[:], in_=z1T[m][:], func=AF.Square,
                                         accum_out=stat1[:, 4 * m + 1:4 * m + 2])

                # ---- AllReduce #1 (z1 batch stats) ----
                st1 = bigp.tile([P, 4], F32)
                nc.vector.tensor_copy(out=st1[:, 0:1], in_=stat1[:, 0:1])
                nc.vector.tensor_copy(out=st1[:, 1:2], in_=stat1[:, 1:2])
                nc.vector.tensor_copy(out=st1[:, 2:3], in_=stat1[:, 4:5])
                nc.vector.tensor_copy(out=st1[:, 3:4], in_=stat1[:, 5:6])
                nc.gpsimd.dma_start(out=cc1_in[:], in_=st1[:])
                if skip_cc:
                    nc.gpsimd.dma_start(out=cc1_out[:], in_=cc1_in[:])
                else:
                    nc.gpsimd.collective_compute(
                        "AllReduce", ALU.add, replica_groups=[list(range(N_CORES))],
                        ins=[cc1_in[:]], outs=[cc1_out[:]])
                ar1 = bigp.tile([P, 4], F32)
                nc.gpsimd.dma_start(out=ar1[:], in_=cc1_out[:])

                # BN1 constants per m-tile: A = g/std, C = be - m*A
                bn1A = bigp.tile([P, 2], F32)
                bn1C = bigp.tile([P, 2], F32)
                tmp = bigp.tile([P, 2], F32)
                mean1 = bigp.tile([P, 2], F32)
                # mean = s/B ; E2 = sq/B ; var = E2 - mean^2
                nc.vector.tensor_scalar(out=mean1[:, 0:1], in0=ar1[:, 0:1],
                                        scalar1=1.0 / B, scalar2=None, op0=ALU.mult)
                nc.vector.tensor_scalar(out=mean1[:, 1:2], in0=ar1[:, 2:3],
                                        scalar1=1.0 / B, scalar2=None, op0=ALU.mult)
                nc.vector.tensor_scalar(out=tmp[:, 0:1], in0=ar1[:, 1:2],
                                        scalar1=1.0 / B, scalar2=None, op0=ALU.mult)
                nc.vector.tensor_scalar(out=tmp[:, 1:2], in0=ar1[:, 3:4],
                                        scalar1=1.0 / B, scalar2=None, op0=ALU.mult)
                msq = bigp.tile([P, 2], F32)
                nc.vector.tensor_tensor(out=msq[:], in0=mean1[:], in1=mean1[:], op=ALU.mult)
                nc.vector.tensor_tensor(out=tmp[:], in0=tmp[:], in1=msq[:], op=ALU.subtract)
                # std = sqrt(var+eps); A = g * (1/std)
                nc.vector.tensor_scalar(out=tmp[:], in0=tmp[:], scalar1=BN_EPS,
                                        scalar2=None, op0=ALU.add)
                nc.scalar.activation(out=tmp[:], in_=tmp[:], func=AF.Sqrt)
                nc.vector.reciprocal(out=tmp[:], in_=tmp[:])
                nc.vector.tensor_tensor(out=bn1A[:], in0=vec1_t[:, 0:2], in1=tmp[:], op=ALU.mult)
                nc.vector.tensor_tensor(out=tmp[:], in0=mean1[:], in1=bn1A[:], op=ALU.mult)
                nc.vector.tensor_tensor(out=bn1C[:], in0=vec1_t[:, 2:4], in1=tmp[:], op=ALU.subtract)
                # fold b1 into C: C += A*b1   (z1 here excludes b1; BN(z+b1) = A*(z+b1)+C')
                nc.vector.tensor_tensor(out=tmp[:], in0=bn1A[:], in1=vec1_t[:, 4:6], op=ALU.mult)
                nc.vector.tensor_tensor(out=bn1C[:], in0=bn1C[:], in1=tmp[:], op=ALU.add)

                # ---- a1 = relu(A*z1 + C), in place ----
                for m in range(2):
                    nc.scalar.activation(out=z1T[m][:], in_=z1T[m][:], func=AF.Relu,
                                         scale=bn1A[:, m:m + 1], bias=bn1C[:, m:m + 1])

                # ---- layer 2 ----
                for g in range(NGR):
                    pz = ps_z.tile([P, GW], F32, tag="pz")
                    for k in range(2):
                        nc.tensor.matmul(out=pz[:], lhsT=w2_t[:, k, :],
                                         rhs=z1T[k][:, g * GW:(g + 1) * GW],
                                         start=(k == 0), stop=(k == 1))
                    nc.vector.tensor_copy(out=z2T[:, g * GW:(g + 1) * GW], in_=pz[:])
                nc.vector.tensor_reduce(out=stat2[:, 0:1], in_=z2T[:],
                                        axis=mybir.AxisListType.X, op=ALU.add)
                nc.scalar.activation(out=sq_scr[:], in_=z2T[:], func=AF.Square,
                                     accum_out=stat2[:, 1:2])

                # ---- AllReduce #2 ----
                nc.gpsimd.dma_start(out=cc2_in[:], in_=stat2[:, 0:2])
                if skip_cc:
                    nc.gpsimd.dma_start(out=cc2_out[:], in_=cc2_in[:])
                else:
                    nc.gpsimd.collective_compute(
                        "AllReduce", ALU.add, replica_groups=[list(range(N_CORES))],
                        ins=[cc2_in[:]], outs=[cc2_out[:]])
                ar2 = bigp.tile([P, 2], F32)
                nc.gpsimd.dma_start(out=ar2[:], in_=cc2_out[:])

                bn2A = bigp.tile([P, 1], F32)
                bn2C = bigp.tile([P, 1], F32)
                m2t = bigp.tile([P, 1], F32)
                v2t = bigp.tile([P, 1], F32)
                nc.vector.tensor_scalar(out=m2t[:], in0=ar2[:, 0:1], scalar1=1.0 / B,
                                        scalar2=None, op0=ALU.mult)
                nc.vector.tensor_scalar(out=v2t[:], in0=ar2[:, 1:2], scalar1=1.0 / B,
                                        scalar2=None, op0=ALU.mult)
                msq2 = bigp.tile([P, 1], F32)
                nc.vector.tensor_tensor(out=msq2[:], in0=m2t[:], in1=m2t[:], op=ALU.mult)
                nc.vector.tensor_tensor(out=v2t[:], in0=v2t[:], in1=msq2[:], op=ALU.subtract)
                nc.vector.tensor_scalar(out=v2t[:], in0=v2t[:], scalar1=BN_EPS,
                                        scalar2=None, op0=ALU.add)
                nc.scalar.activation(out=v2t[:], in_=v2t[:], func=AF.Sqrt)
                nc.vector.reciprocal(out=v2t[:], in_=v2t[:])
                nc.vector.tensor_tensor(out=bn2A[:], in0=vec2_t[:, 0:1], in1=v2t[:], op=ALU.mult)
                nc.vector.tensor_tensor(out=m2t[:], in0=m2t[:], in1=bn2A[:], op=ALU.mult)
                nc.vector.tensor_tensor(out=bn2C[:], in0=vec2_t[:, 1:2], in1=m2t[:], op=ALU.subtract)
                nc.vector.tensor_tensor(out=msq2[:], in0=bn2A[:], in1=vec2_t[:, 2:3], op=ALU.mult)
                nc.vector.tensor_tensor(out=bn2C[:], in0=bn2C[:], in1=msq2[:], op=ALU.add)

                nc.scalar.activation(out=z2T[:], in_=z2T[:], func=AF.Relu,
                                     scale=bn2A[:], bias=bn2C[:])

                # ---- layer 3 + output ----
                for t in range(NT):
                    pz3 = ps_s.tile([P, 1], F32, tag="pz3")
                    nc.tensor.matmul(out=pz3[:], lhsT=z2T[:, t * P:(t + 1) * P],
                                     rhs=w3_t[:], start=True, stop=True)
                    nc.vector.tensor_tensor(out=out_sb[:, t:t + 1], in0=pz3[:],
                                            in1=fm_sb[:, t:t + 1], op=ALU.add)
                nc.sync.dma_start(out=out[:], in_=out_sb[:])

    nc.compile()
    return nc


def _prep_inputs(x, emb_table, fm1_table, dense_w, dense_b, dense_fm_w,
                 dense_fm_b, W1, b1, g1, be1, W2, b2, g2, be2, W3, b3):
    tbl = np.concatenate([np.asarray(emb_table, np.float32),
                          np.asarray(fm1_table, np.float32)], axis=1)
    # W1 [528,256] -> padded chunks [5,128,256]
    bounds = [0, 128, 256, 320, 448, 528]
    w1p = np.zeros((5, P, H1), np.float32)
    W1 = np.asarray(W1, np.float32)
    for k in range(5):
        n = bounds[k + 1] - bounds[k]
        w1p[k, :n] = W1[bounds[k]:bounds[k + 1]]
    w2p = np.asarray(W2, np.float32).reshape(2, P, H2)
    w3p = np.asarray(W3, np.float32).reshape(P, 1)
    # vec1: [128, 8]: cols 0-1 g1 (m-tiles), 2-3 be1, 4-5 b1, 6-7 pad
    v1 = np.zeros((P, 8), np.float32)
    v1[:, 0:2] = np.asarray(g1, np.float32).reshape(2, P).T
    v1[:, 2:4] = np.asarray(be1, np.float32).reshape(2, P).T
    v1[:, 4:6] = np.asarray(b1, np.float32).reshape(2, P).T
    v2 = np.zeros((P, 4), np.float32)
    v2[:, 0] = np.asarray(g2, np.float32)
    v2[:, 1] = np.asarray(be2, np.float32)
    v2[:, 2] = np.asarray(b2, np.float32)
    dvec = np.zeros((1, 2 * ND * FEAT + ND + 2), np.float32)
    dvec[0, 0:ND * FEAT] = np.asarray(dense_w, np.float32).reshape(-1)
    dvec[0, ND * FEAT:2 * ND * FEAT] = np.asarray(dense_b, np.float32).reshape(-1)
    dvec[0, 2 * ND * FEAT:2 * ND * FEAT + ND] = np.asarray(dense_fm_w, np.float32)
    dvec[0, 2 * ND * FEAT + ND] = float(np.sum(dense_fm_b)) + float(np.asarray(b3).reshape(-1)[0])
    offs = np.repeat(OFFSETS.reshape(1, NS), P, axis=0)
    x = np.asarray(x, np.float32)
    in_maps = []
    for c in range(N_CORES):
        in_maps.append({
            "xs": x[c * BC:(c + 1) * BC],
            "tbl": tbl, "offs": offs, "w1": w1p, "w2": w2p, "w3": w3p,
            "vec1": v1, "vec2": v2, "dvec": np.repeat(dvec, P, axis=0),
            "iota_d": np.arange(P, dtype=np.float32).reshape(P, 1),
            "ones_d": np.ones((1, P), np.float32),
        })
    return in_maps


def kernel(**inputs) -> np.ndarray:
    if "nc" not in _CACHE:
        _CACHE["nc"] = _build()
    nc = _CACHE["nc"]
    in_maps = _prep_inputs(**inputs)
    res = run_bass_kernel_spmd(nc, in_maps, core_ids=list(range(N_CORES)))
    y = np.empty((B, 1), np.float32)
    for c in range(N_CORES):
        o = res.results[c]["out"]          # [P, NT]: sample t*128+p at [p, t]
        y[c * BC:(c + 1) * BC, 0] = o.T.reshape(-1)
    return y

